# revision 19
# baseline (speedup 1.0000x reference)
"""Trainium2 Bass kernel for nn_DetectionLoss (YOLO-style detection loss).

Strategy (data parallel over batch, 8 cores x 2 images):

The loss decomposes into (a) masked reductions over positive cells (GIoU +
cls BCE, 32 cells/image, host-gathered rows), (b) a dense obj-BCE over all
19200 cells/image, and (c) an ignore-mask correction.  The key identity for
(c): IoU(pred_i, gt_k) > 0.5 requires |cx_i-CX_k| < min(hw_i, HW_k) <= HW_k
(and same in y), so the only cells that can ever be ignored lie in small
host-computable rectangles around each GT box -- ~2.5k cells/image instead
of 19200.  The host emits a flat (candidate-cell, gt) pair list (R=4 slots
per candidate, max multiplicity seen is 3); the device evaluates the exact
interval-overlap IoU test per pair in ~16 elementwise ops at FD=168 total
(instead of a 32-iteration loop over full planes), reduces over slots, and
corrects the dense negative-BCE sum:
    neg_obj = sum(spo) - sum(spo*tobj) - sum(spo*ign*(1-tobj))
    n_neg   = (19200-32)*B - sum(ign*(1-tobj))
Overlap per axis is computed in the exact interval form
    iw = min(cx+hw, CX+HW) - max(cx-hw, CX-HW)
with the grid offset folded host-side into the GT edges (XR/XL), working in
160th-of-image units so cx = tanh(tx/2) + const and hw = exp(tw + ln(aw/8)).

Engine split: pair chain + reductions on DVE, transcendentals on ACT (one
table-set switch), the positive-cell GIoU block on the otherwise-idle
GpSimd engine, inputs streamed over four parallel HWDGE rings + SWDGE.
Per-core partial sums land in one [1,16] vector; host combines.
"""
import os
import sys
import types

import numpy as np

# ---- axon NTFF profiling hook (missing antenv.axon_hooks in this image) ----
try:
    import antenv

    if "antenv.axon_hooks" not in sys.modules:
        _m = types.ModuleType("antenv.axon_hooks")
        _m._hook = None
        _m.set_axon_ntff_profile_hook = lambda h: setattr(_m, "_hook", h)
        _m.get_axon_ntff_profile_hook = lambda: _m._hook
        sys.modules["antenv.axon_hooks"] = _m
        antenv.axon_hooks = _m
        try:
            from trn_agent_boot.trn_boot import _ntff_profile_via_ctypes

            _m.set_axon_ntff_profile_hook(
                _ntff_profile_via_ctypes("/opt/axon/libaxon_pjrt.so")
            )
        except Exception:
            pass
except Exception:
    pass

import concourse.bass as bass
import concourse.bass_utils as bass_utils
import concourse.mybir as mybir
import concourse.tile as tile_mod
from concourse.tile_rust import add_dep_helper
from concourse.vector_clock import ScopedClock

# No bucket creds in this container; keep trace artifacts local.
bass_utils.upload_artifacts = lambda tmpdir: tmpdir


# ---- workaround: this walrus build rejects >2 sync waits on one CTRL ----
def _patched_drain_and_barrier(self, tick_clock, wait_clock):
    nc = self.nc
    probe = nc.sync.nop(nofuse=True)
    wait_clock.add_sem_waits(probe.ins, ScopedClock({None: tick_clock.global_clock}))
    si = probe.ins.sync_info
    waits = list(si.on_wait or [])
    if len(waits) > 1:
        si.on_wait = waits[:1]
        for w in waits[1:]:
            extra = nc.sync.nop(nofuse=True)
            extra.ins.sync_info = mybir.SyncInfo(on_wait=[w], on_update=[])
    nc.sync.drain()
    nc.all_engine_barrier()
    assert self.sems is not None
    popped = nc._tile_sem_poison_stack.pop()
    assert popped is self._sem_poison
    nc.clear_and_free_semaphores(list(self.sems.allocated().values()))
    nc.all_engine_barrier()


tile_mod.TileContext._drain_and_barrier = _patched_drain_and_barrier


def _split_sync_waits(nc, limit=1):
    """Split >limit sem waits per instruction onto preceding same-engine NoOps
    (this walrus build rejects instructions with more sync waits)."""
    for fn in nc.m.functions:
        for bb in fn.blocks:
            newlist = []
            for ins in bb.instructions:
                si = ins.sync_info
                waits = list(si.on_wait or []) if si is not None else []
                if len(waits) > limit:
                    si.on_wait = waits[:limit]
                    extra = waits[limit:]
                    for i in range(0, len(extra), limit):
                        newlist.append(mybir.InstNoOp(
                            name=f"{ins.name}-waitsplit{i}",
                            engine=ins.engine,
                            ins=[],
                            outs=[],
                            sync_info=mybir.SyncInfo(
                                on_wait=extra[i:i + limit], on_update=[]),
                        ))
                newlist.append(ins)
            bb.instructions = newlist

# ---- problem constants (hardcoded; kernel.py must be self-contained) ----
B, A, H, W = 16, 3, 80, 80
C = 85
CELLS = A * H * W          # 19200
M = 32                     # positives per image
EPS = 1e-8
ANCHORS = np.array([[10.0, 13.0], [16.0, 30.0], [33.0, 23.0]], np.float32)
NCORES = 8
BPC = B // NCORES          # 2 images per core
P = 128
T = BPC * CELLS // P       # 300 free-dim cells per partition
HP = P // BPC              # 64 partitions per image

CAND_N = 5120              # candidate slots per core (2 images), = 128*40
CFD = CAND_N // P          # 40
R = 3                      # gt slots per candidate (max multiplicity is 3)
PFD = CFD * R              # 120
POSG = 178                 # pos row width (packed 4 rows/partition)
PP = 16                    # pos partitions (64 rows / 4)

F32 = mybir.dt.float32
AF = mybir.ActivationFunctionType
OP = mybir.AluOpType

LAST_EXEC_NS = None
LAST_RESULT = None
_NC_CACHE = None

# pk channel indices: pka = [PTX, PTY], pkb = [PTWL, PTHL],
# pkc = [XRX, XLX, XRY, XLY, CK3P]


def _build_nc():
    nc = bass.Bass("TRN2", target_bir_lowering=False, debug=False)
    big_t = nc.dram_tensor("big", [P, 9 * PFD + 2 * T + 2 * CFD], F32,
                           kind="ExternalInput").ap()
    pos_t = nc.dram_tensor("pos", [PP, 4 * POSG], F32, kind="ExternalInput").ap()
    out_t = nc.dram_tensor("out", [1, 16], F32, kind="ExternalOutput").ap()

    with tile_mod.TileContext(nc) as tc:
        _body(nc, tc, big_t, pos_t, out_t)
    _split_sync_waits(nc)
    return nc


def _body(nc, tc, big_t, pos_t, out_t):
    from contextlib import ExitStack

    ctx = ExitStack()
    with ctx:
        const = ctx.enter_context(tc.tile_pool(name="const", bufs=1))
        work = ctx.enter_context(tc.tile_pool(name="work", bufs=1))
        psum = ctx.enter_context(tc.tile_pool(name="psum", bufs=1, space="PSUM"))

        # ---------- DMAs: everything per-partition-contiguous in ONE fat
        # tensor (DMA cost is ~30ns/packet + bytes/BW; one 7KB row per
        # partition per ring-half minimizes packets), halves split over the
        # two HWDGE rings; pos rides SWDGE ----------
        posf = const.tile([PP, 4, POSG], F32)
        nc.sync.dma_start(out=posf[:].rearrange("p a g -> p (a g)"), in_=pos_t)
        pos = posf[:]
        SK = 68  # sync ring starts ~1us earlier; give it a few more rows
        big = const.tile([P, 9 * PFD + 2 * T + 2 * CFD], F32)
        nc.sync.dma_start(out=big[:SK], in_=big_t[:SK])
        nc.scalar.dma_start(out=big[SK:], in_=big_t[SK:])
        pk = big[:, 0:9 * PFD].rearrange("p (c f) -> p c f", c=9)

        xo = big[:, 9 * PFD:9 * PFD + T]
        xoc = big[:, 9 * PFD + 2 * T:9 * PFD + 2 * T + CFD]
        npc = big[:, 9 * PFD + 2 * T + CFD:9 * PFD + 2 * T + 2 * CFD]

        stats = const.tile([P, 16], F32)
        nc.vector.memset(stats[:], 0.0)
        # cols: 0 giou, 1 spc, 2 pts, 3 sc1, 4 sc2, 5 spoA, 6 corrS, 7 corrN

        # ---------- ACT stream: pos-block transcendentals first (pos lands
        # ~3us before the big tensor), then the pair/plane exps, then ONE
        # table-set switch for the Ln ops ----------
        # s64 writes tanh(ptxy/2) into pos cols 6:8 (host left them zero), so
        # cxy_pt below reads one contiguous block.
        nc.scalar.activation(pos[:, :, 6:8], pos[:, :, 0:2], AF.Tanh,
                             scale=0.5)
        e_pt = work.tile([PP, 4, 4], F32)
        nc.scalar.activation(e_pt[:], pos[:, :, 2:6], AF.Exp)
        ec = work.tile([PP, 4, 80], F32)
        nc.scalar.activation(ec[:], pos[:, :, 98:178], AF.Exp)
        ep1 = work.tile([PP, 4, 1], F32)
        nc.scalar.activation(ep1[:], pos[:, :, 14:15], AF.Exp)
        thxy = work.tile([P, 2, PFD], F32)
        nc.scalar.activation(thxy[:], pk[:, 0:2, :], AF.Tanh, scale=0.5)
        thx = thxy[:, 0, :]
        thy = thxy[:, 1, :]
        ewh2 = work.tile([P, 2, PFD], F32)
        nc.scalar.activation(ewh2[:], pk[:, 2:4, :], AF.Exp)
        ew = ewh2[:, 0, :]
        eh = ewh2[:, 1, :]
        eo = work.tile([P, T], F32)
        nc.scalar.activation(eo[:], xo, AF.Exp)
        eoc = work.tile([P, CFD], F32)
        nc.scalar.activation(eoc[:], xoc, AF.Exp)
        # (natural_log_exp set loads here, before the first Ln)
        spoc = work.tile([P, CFD], F32)
        nc.scalar.activation(spoc[:], eoc[:], AF.Ln, bias=1.0)
        spo = work.tile([P, T], F32)
        nc.scalar.activation(spo[:], eo[:], AF.Ln, bias=1.0,
                             accum_out=stats[:, 5:6])
        sp1 = work.tile([PP, 4, 1], F32)
        nc.scalar.activation(sp1[:], ep1[:], AF.Ln, bias=1.0,
                             accum_out=stats[0:PP, 3:4])
        spc = work.tile([PP, 4, 80], F32)
        nc.scalar.activation(spc[:], ec[:], AF.Ln, bias=1.0,
                             accum_out=stats[0:PP, 1:2])

        # ---------- pair chain: front arithmetic on Pool (free there once
        # the pos block drains), comparisons + the rest on DVE ----------
        sx = work.tile([P, PFD], F32)
        nc.gpsimd.tensor_add(sx[:], thx, ew)
        dxm = work.tile([P, PFD], F32)
        nc.gpsimd.tensor_sub(dxm[:], thx, ew)
        sy = work.tile([P, PFD], F32)
        nc.gpsimd.tensor_add(sy[:], thy, eh)
        dym = work.tile([P, PFD], F32)
        nc.gpsimd.tensor_sub(dym[:], thy, eh)
        nh34 = work.tile([P, PFD], F32)
        nc.gpsimd.tensor_mul(nh34[:], ew, eh)
        nh3 = work.tile([P, PFD], F32)
        nc.gpsimd.tensor_scalar_mul(nh3[:], nh34[:], -4.0 / 3.0)
        r1x = work.tile([P, PFD], F32)
        nc.vector.tensor_tensor(r1x[:], sx[:], pk[:, 4, :], op=OP.min)
        r2x = work.tile([P, PFD], F32)
        nc.vector.tensor_max(r2x[:], dxm[:], pk[:, 5, :])
        r1y = work.tile([P, PFD], F32)
        nc.vector.tensor_tensor(r1y[:], sy[:], pk[:, 6, :], op=OP.min)
        r2y = work.tile([P, PFD], F32)
        nc.vector.tensor_max(r2y[:], dym[:], pk[:, 7, :])
        mth = work.tile([P, PFD], F32)
        nc.vector.tensor_sub(mth[:], nh3[:], pk[:, 8, :])
        nx = work.tile([P, PFD], F32)
        nc.vector.tensor_sub(nx[:], r2x[:], r1x[:])
        ny = work.tile([P, PFD], F32)
        nc.vector.tensor_sub(ny[:], r2y[:], r1y[:])
        rh = work.tile([P, PFD], F32)
        nc.vector.tensor_scalar(
            out=rh[:], in0=ny[:], scalar1=-1.0, scalar2=0.0,
            op0=OP.mult, op1=OP.max)
        ni = work.tile([P, PFD], F32)
        nc.vector.scalar_tensor_tensor(
            out=ni[:], in0=nx[:], scalar=0.0, in1=rh[:],
            op0=OP.min, op1=OP.mult)
        ind = work.tile([P, CFD, R], F32)
        i_ind = nc.vector.tensor_tensor(
            ind[:].rearrange("p c r -> p (c r)"), ni[:], mth[:], op=OP.is_lt)
        ign = work.tile([P, CFD], F32)
        nc.vector.tensor_reduce(
            out=ign[:], in_=ind[:], axis=mybir.AxisListType.X, op=OP.max)

        # ---------- candidate corrections ----------
        t1 = work.tile([P, CFD], F32)
        nc.vector.scalar_tensor_tensor(
            out=t1[:], in0=ign[:], scalar=1.0, in1=npc,
            op0=OP.mult, op1=OP.mult, accum_out=stats[:, 7:8])
        t2 = work.tile([P, CFD], F32)
        nc.vector.scalar_tensor_tensor(
            out=t2[:], in0=t1[:], scalar=1.0, in1=spoc[:],
            op0=OP.mult, op1=OP.mult, accum_out=stats[:, 6:7])

        # ---------- positive-cell obj sums straight from the pos rows ----
        xp = work.tile([PP, 4, 1], F32)
        nc.vector.tensor_scalar(
            out=xp[:], in0=pos[:, :, 14:15], scalar1=1.0, scalar2=0.0,
            op0=OP.mult, op1=OP.add, accum_out=stats[0:PP, 4:5])

        # ---------- positive-cell block on GpSimd (idle engine) ----------
        # Works in 160th-of-image units with host-folded anchors, so e_pt IS
        # the half-width vector [hwp, hhp, hwt, hht]; areas are tracked in
        # quarter-units (hw*hh = area/4) which cancels in GIoU once EPS is
        # scaled by 160^2/4 = 6400.  Pool has no scalar_tensor_tensor, so
        # everything is tensor_tensor / tensor_scalar.
        g = nc.gpsimd
        cxy = work.tile([PP, 4, 4], F32)
        g.tensor_add(cxy[:], pos[:, :, 6:10], pos[:, :, 10:14])
        x1 = work.tile([PP, 4, 4], F32)
        g.tensor_sub(x1[:], cxy[:], e_pt[:])
        x2 = work.tile([PP, 4, 4], F32)
        g.tensor_add(x2[:], cxy[:], e_pt[:])
        # (Pool TT has no max/min -- these four comparisons run on DVE)
        imax = work.tile([PP, 4, 2], F32)
        nc.vector.tensor_max(imax[:], x1[:, :, 0:2], x1[:, :, 2:4])
        imin = work.tile([PP, 4, 2], F32)
        nc.vector.tensor_tensor(imin[:], x2[:, :, 0:2], x2[:, :, 2:4],
                                op=OP.min)
        emin = work.tile([PP, 4, 2], F32)
        nc.vector.tensor_tensor(emin[:], x1[:, :, 0:2], x1[:, :, 2:4],
                                op=OP.min)
        emax = work.tile([PP, 4, 2], F32)
        nc.vector.tensor_max(emax[:], x2[:, :, 0:2], x2[:, :, 2:4])
        iwh = work.tile([PP, 4, 2], F32)
        g.tensor_sub(iwh[:], imin[:], imax[:])
        iwr = work.tile([PP, 4, 2], F32)
        g.tensor_scalar(out=iwr[:], in0=iwh[:], scalar1=0.5, scalar2=0.0,
                        op0=OP.mult, op1=OP.max)
        inter = work.tile([PP, 4, 1], F32)
        g.tensor_mul(inter[:], iwr[:, :, 0:1], iwr[:, :, 1:2])   # inter/4
        aprod = work.tile([PP, 4, 2], F32)
        g.tensor_mul(aprod[:], e_pt[:, :, 0:4:2], e_pt[:, :, 1:4:2])
        uae = work.tile([PP, 4, 2], F32)
        g.tensor_add(uae[:, :, 0:1], aprod[:, :, 0:1], aprod[:, :, 1:2])
        g.tensor_sub(uae[:, :, 0:1], uae[:, :, 0:1], inter[:])   # union/4
        ewh = work.tile([PP, 4, 2], F32)
        g.tensor_sub(ewh[:], emax[:], emin[:])
        ewh5 = work.tile([PP, 4, 2], F32)
        g.tensor_scalar_mul(ewh5[:], ewh[:], 0.5)
        g.tensor_mul(uae[:, :, 1:2], ewh5[:, :, 0:1], ewh5[:, :, 1:2])
        dcu = work.tile([PP, 4, 1], F32)
        g.tensor_sub(dcu[:], uae[:, :, 1:2], uae[:, :, 0:1])
        uaeE = work.tile([PP, 4, 2], F32)
        g.tensor_scalar_add(uaeE[:], uae[:], 6400.0 * EPS)

        rr = work.tile([PP, 4, 2], F32)
        nc.vector.reciprocal(rr[:], uaeE[:])
        # giou_sum = n_pos - sum(iou) + sum(qv); host folds the constant
        iou = work.tile([PP, 4, 1], F32)
        nc.vector.scalar_tensor_tensor(
            out=iou[:], in0=inter[:], scalar=1.0, in1=rr[:, :, 0:1],
            op0=OP.mult, op1=OP.mult, accum_out=stats[0:PP, 0:1])
        qv = work.tile([PP, 4, 1], F32)
        nc.vector.scalar_tensor_tensor(
            out=qv[:], in0=dcu[:], scalar=1.0, in1=rr[:, :, 1:2],
            op0=OP.mult, op1=OP.mult, accum_out=stats[0:PP, 8:9])
        ptS = work.tile([PP, 4, 80], F32)
        nc.vector.scalar_tensor_tensor(
            out=ptS[:], in0=pos[:, :, 98:178], scalar=1.0,
            in1=pos[:, :, 18:98],
            op0=OP.mult, op1=OP.mult, accum_out=stats[0:PP, 2:3])

        # ---------- final partition reduction + output ----------
        ones = const.tile([P, 1], F32)
        nc.vector.memset(ones[:], 1.0)
        pst = psum.tile([1, 16], F32)
        nc.tensor.matmul(pst[:], ones[:], stats[:], start=True, stop=True)
        res = const.tile([1, 16], F32)
        nc.scalar.copy(res[:], pst[:])
        nc.sync.dma_start(out=out_t, in_=res[:])


def _host_prep(preds, targets):
    """Build per-core input maps from the full inputs (indexing/layout only)."""
    preds = np.ascontiguousarray(preds, np.float32)
    targets = np.ascontiguousarray(targets, np.float32)
    assert preds.shape == (B, A, H, W, C), preds.shape

    j = np.arange(CELLS)
    a = j // (H * W)
    rem = j % (H * W)
    gy = (rem // W).astype(np.float32)
    gx = (rem % W).astype(np.float32)
    aw = ANCHORS[a, 0]
    ah = ANCHORS[a, 1]
    lnaw8 = np.log(aw / 8.0).astype(np.float32)
    lnah8 = np.log(ah / 8.0).astype(np.float32)

    pf = preds.reshape(B, CELLS, C)
    tf = targets.reshape(B, CELLS, C)
    HW2 = H * W

    def plane2(x0, x1):
        return np.concatenate([x0.reshape(HP, T), x1.reshape(HP, T)], 0)

    in_maps = []
    for c in range(NCORES):
        i0 = BPC * c
        NP = CAND_N * R
        pk = np.zeros((9, NP), np.float32)
        pk[8] = 1e30
        ckv = np.zeros((2, CAND_N), np.float32)
        pos = np.zeros((2 * M, POSG), np.float32)

        ci_base = 0
        for ii in range(BPC):
            b = i0 + ii
            to = tf[b, :, 4]
            idx = np.nonzero(to > 0)[0]
            assert len(idx) == M, len(idx)
            tb = tf[b][idx]
            tb64 = tb.astype(np.float64)
            gxk = gx[idx].astype(np.float64)
            gyk = gy[idx].astype(np.float64)
            CX160 = 2.0 * (tb64[:, 0] + gxk)
            CY160 = 2.0 * (tb64[:, 1] + gyk)
            HW160 = aw[idx] * np.exp(tb64[:, 2]) / 8.0
            HH160 = ah[idx] * np.exp(tb64[:, 3]) / 8.0
            CK3 = (4.0 * HW160 * HH160 + 25600.0 * EPS) / 3.0

            # candidate rectangles (2D grid), multiplicity and slot layout
            CXn = CX160 / 160.0
            CYn = CY160 / 160.0
            HWn = HW160 / 160.0
            HHn = HH160 / 160.0
            x0r = np.maximum(0, np.floor(80 * (CXn - HWn)).astype(np.int64))
            x1r = np.minimum(W - 1, np.ceil(80 * (CXn + HWn)).astype(np.int64))
            y0r = np.maximum(0, np.floor(80 * (CYn - HHn)).astype(np.int64))
            y1r = np.minimum(H - 1, np.ceil(80 * (CYn + HHn)).astype(np.int64))
            mult = np.zeros(HW2, np.int64)
            rec_cell = []
            rec_k = []
            rec_slot = []
            for k in range(M):
                yy, xx = np.meshgrid(
                    np.arange(y0r[k], y1r[k] + 1),
                    np.arange(x0r[k], x1r[k] + 1), indexing="ij")
                cells2d = (yy * W + xx).ravel()
                rec_cell.append(cells2d)
                rec_k.append(np.full(len(cells2d), k, np.int64))
                rec_slot.append(mult[cells2d].copy())
                mult[cells2d] += 1
            rec_cell = np.concatenate(rec_cell)
            rec_k = np.concatenate(rec_k)
            rec_slot = np.concatenate(rec_slot)
            assert mult.max() <= R, mult.max()

            cand2d = np.nonzero(mult > 0)[0]
            ncand2d = len(cand2d)
            crank = np.full(HW2, -1, np.int64)
            crank[cand2d] = np.arange(ncand2d)
            assert ci_base + 3 * ncand2d <= CAND_N

            for aa in range(3):
                ci = ci_base + aa * ncand2d + np.arange(ncand2d)
                cells = aa * HW2 + cand2d
                ckv[0, ci] = pf[b, cells, 4]
                ckv[1, ci] = 1.0 - tf[b, cells, 4]

            for aa in range(3):
                ci = ci_base + aa * ncand2d + crank[rec_cell]
                pidx = ci * R + rec_slot
                cells = aa * HW2 + rec_cell
                g1x = 2.0 * gx[cells] + 1.0
                g1y = 2.0 * gy[cells] + 1.0
                pk[0, pidx] = pf[b, cells, 0]
                pk[1, pidx] = pf[b, cells, 1]
                pk[2, pidx] = pf[b, cells, 2] + lnaw8[cells]
                pk[3, pidx] = pf[b, cells, 3] + lnah8[cells]
                pk[4, pidx] = (CX160[rec_k] + HW160[rec_k]) - g1x
                pk[5, pidx] = (CX160[rec_k] - HW160[rec_k]) - g1x
                pk[6, pidx] = (CY160[rec_k] + HH160[rec_k]) - g1y
                pk[7, pidx] = (CY160[rec_k] - HH160[rec_k]) - g1y
                pk[8, pidx] = CK3[rec_k]
            ci_base += 3 * ncand2d

            # positive rows (packed layout in 160-units, see _body)
            r = slice(M * ii, M * (ii + 1))
            pos[r, 0:2] = pf[b][idx][:, 0:2]              # pred tx, ty
            pos[r, 2] = pf[b][idx][:, 2] + lnaw8[idx]     # exp -> hwp160
            pos[r, 3] = pf[b][idx][:, 3] + lnah8[idx]
            pos[r, 4] = tb[:, 2] + lnaw8[idx]             # exp -> hwt160
            pos[r, 5] = tb[:, 3] + lnah8[idx]
            # cols 6:8 left zero (device writes tanh there)
            pos[r, 8:10] = 2.0 * tb[:, 0:2]               # 2*ttx, 2*tty
            pos[r, 10] = 2.0 * gx[idx] + 1.0
            pos[r, 11] = 2.0 * gy[idx] + 1.0
            pos[r, 12] = 2.0 * gx[idx]
            pos[r, 13] = 2.0 * gy[idx]
            pos[r, 14] = pf[b][idx][:, 4]                 # obj logit
            pos[r, 18:98] = tb[:, 5:85]                   # tgt cls
            pos[r, 98:178] = pf[b][idx][:, 5:85]          # pred cls logits

        plane = np.concatenate([
            plane2(pf[i0, :, 4], pf[i0 + 1, :, 4]),
            plane2(tf[i0, :, 4], tf[i0 + 1, :, 4]),
            ckv[0].reshape(P, CFD),
            ckv[1].reshape(P, CFD),
        ], axis=1).astype(np.float32)  # [128, 684]

        big = np.concatenate(
            [pk.reshape(9, P, PFD).transpose(1, 0, 2).reshape(P, 9 * PFD),
             plane], axis=1)
        in_maps.append({
            "big": np.ascontiguousarray(big),
            "pos": np.ascontiguousarray(pos.reshape(PP, 4 * POSG)),
        })
    return in_maps


def _combine(outs):
    s = np.sum(np.stack([o["out"].ravel() for o in outs]), axis=0,
               dtype=np.float64)
    n_pos = float(B * M)
    giou_val = (n_pos - s[0] + s[8]) / (n_pos + EPS)
    cls_val = (s[1] - s[2]) / (n_pos + EPS)
    pos_obj = s[3] - s[4]
    neg_obj = (s[5] - s[3]) - s[6]
    n_neg = B * (CELLS - M) - s[7]
    obj_val = (5.0 * pos_obj + neg_obj) / (5.0 * n_pos + n_neg + EPS)
    total = giou_val + obj_val + cls_val
    return np.array([total, giou_val, obj_val, cls_val], np.float32)


def kernel(preds, targets):
    global LAST_EXEC_NS, LAST_RESULT, _NC_CACHE
    in_maps = _host_prep(preds, targets)
    if _NC_CACHE is None:
        _NC_CACHE = _build_nc()
    nc = _NC_CACHE
    trace = os.environ.get("CCK_TRACE") == "1"
    res = None
    if trace:
        try:
            res = bass_utils.run_bass_kernel_spmd(
                nc, in_maps, core_ids=list(range(NCORES)), trace=True)
            LAST_EXEC_NS = res.exec_time_ns
        except Exception as e:
            print(f"[kernel] traced run failed ({e!r}); retrying untraced",
                  file=sys.stderr)
            res = None
    if res is None:
        res = bass_utils.run_bass_kernel_spmd(
            nc, in_maps, core_ids=list(range(NCORES)), trace=False)
    LAST_RESULT = res
    return _combine(res.results)


# revision 21
# speedup vs baseline: 1.4174x; 1.4174x over previous
"""Trainium2 Bass kernel for nn_DetectionLoss (YOLO-style detection loss).

Strategy (data parallel over batch, 8 cores x 2 images):

The loss decomposes into (a) masked reductions over positive cells (GIoU +
cls BCE, 32 cells/image, host-gathered rows), (b) a dense obj-BCE over all
19200 cells/image, and (c) an ignore-mask correction.  The key identity for
(c): IoU(pred_i, gt_k) > 0.5 requires |cx_i-CX_k| < min(hw_i, HW_k) <= HW_k
(and same in y), so the only cells that can ever be ignored lie in small
host-computable rectangles around each GT box -- ~2.5k cells/image instead
of 19200.  The host emits a flat (candidate-cell, gt) pair list (R=4 slots
per candidate, max multiplicity seen is 3); the device evaluates the exact
interval-overlap IoU test per pair in ~16 elementwise ops at FD=168 total
(instead of a 32-iteration loop over full planes), reduces over slots, and
corrects the dense negative-BCE sum:
    neg_obj = sum(spo) - sum(spo*tobj) - sum(spo*ign*(1-tobj))
    n_neg   = (19200-32)*B - sum(ign*(1-tobj))
Overlap per axis is computed in the exact interval form
    iw = min(cx+hw, CX+HW) - max(cx-hw, CX-HW)
with the grid offset folded host-side into the GT edges (XR/XL), working in
160th-of-image units so cx = tanh(tx/2) + const and hw = exp(tw + ln(aw/8)).

Engine split: pair chain + reductions on DVE, transcendentals on ACT (one
table-set switch), the positive-cell GIoU block on the otherwise-idle
GpSimd engine, inputs streamed over four parallel HWDGE rings + SWDGE.
Per-core partial sums land in one [1,16] vector; host combines.
"""
import os
import sys
import types

import numpy as np

# ---- axon NTFF profiling hook (missing antenv.axon_hooks in this image) ----
try:
    import antenv

    if "antenv.axon_hooks" not in sys.modules:
        _m = types.ModuleType("antenv.axon_hooks")
        _m._hook = None
        _m.set_axon_ntff_profile_hook = lambda h: setattr(_m, "_hook", h)
        _m.get_axon_ntff_profile_hook = lambda: _m._hook
        sys.modules["antenv.axon_hooks"] = _m
        antenv.axon_hooks = _m
        try:
            from trn_agent_boot.trn_boot import _ntff_profile_via_ctypes

            _m.set_axon_ntff_profile_hook(
                _ntff_profile_via_ctypes("/opt/axon/libaxon_pjrt.so")
            )
        except Exception:
            pass
except Exception:
    pass

import concourse.bass as bass
import concourse.bass_utils as bass_utils
import concourse.mybir as mybir
import concourse.tile as tile_mod
from concourse.tile_rust import add_dep_helper
from concourse.vector_clock import ScopedClock

# No bucket creds in this container; keep trace artifacts local.
bass_utils.upload_artifacts = lambda tmpdir: tmpdir


# ---- workaround: this walrus build rejects >2 sync waits on one CTRL ----
def _patched_drain_and_barrier(self, tick_clock, wait_clock):
    nc = self.nc
    probe = nc.sync.nop(nofuse=True)
    wait_clock.add_sem_waits(probe.ins, ScopedClock({None: tick_clock.global_clock}))
    si = probe.ins.sync_info
    waits = list(si.on_wait or [])
    if len(waits) > 1:
        si.on_wait = waits[:1]
        for w in waits[1:]:
            extra = nc.sync.nop(nofuse=True)
            extra.ins.sync_info = mybir.SyncInfo(on_wait=[w], on_update=[])
    nc.sync.drain()
    nc.all_engine_barrier()
    assert self.sems is not None
    popped = nc._tile_sem_poison_stack.pop()
    assert popped is self._sem_poison
    nc.clear_and_free_semaphores(list(self.sems.allocated().values()))
    nc.all_engine_barrier()


tile_mod.TileContext._drain_and_barrier = _patched_drain_and_barrier


def _split_sync_waits(nc, limit=1):
    """Split >limit sem waits per instruction onto preceding same-engine NoOps
    (this walrus build rejects instructions with more sync waits)."""
    for fn in nc.m.functions:
        for bb in fn.blocks:
            newlist = []
            for ins in bb.instructions:
                si = ins.sync_info
                waits = list(si.on_wait or []) if si is not None else []
                if len(waits) > limit:
                    si.on_wait = waits[:limit]
                    extra = waits[limit:]
                    for i in range(0, len(extra), limit):
                        newlist.append(mybir.InstNoOp(
                            name=f"{ins.name}-waitsplit{i}",
                            engine=ins.engine,
                            ins=[],
                            outs=[],
                            sync_info=mybir.SyncInfo(
                                on_wait=extra[i:i + limit], on_update=[]),
                        ))
                newlist.append(ins)
            bb.instructions = newlist

# ---- problem constants (hardcoded; kernel.py must be self-contained) ----
B, A, H, W = 16, 3, 80, 80
C = 85
CELLS = A * H * W          # 19200
M = 32                     # positives per image
EPS = 1e-8
ANCHORS = np.array([[10.0, 13.0], [16.0, 30.0], [33.0, 23.0]], np.float32)
NCORES = 8
BPC = B // NCORES          # 2 images per core
P = 128
T = BPC * CELLS // P       # 300 free-dim cells per partition
HP = P // BPC              # 64 partitions per image

CAND_N = 5120              # candidate slots per core (2 images), = 128*40
CFD = CAND_N // P          # 40
R = 3                      # gt slots per candidate (max multiplicity is 3)
PFD = CFD * R              # 120
POSG = 178                 # pos row width (packed 4 rows/partition)
PP = 16                    # pos partitions (64 rows / 4)

F32 = mybir.dt.float32
F16 = mybir.dt.float16
AF = mybir.ActivationFunctionType
OP = mybir.AluOpType

LAST_EXEC_NS = None
LAST_RESULT = None
_NC_CACHE = None

# pk channel indices: pka = [PTX, PTY], pkb = [PTWL, PTHL],
# pkc = [XRX, XLX, XRY, XLY, CK3P]


def _build_nc():
    nc = bass.Bass("TRN2", target_bir_lowering=False, debug=False)
    big_t = nc.dram_tensor("big", [P, 9 * PFD + 2 * T + 2 * CFD], F16,
                           kind="ExternalInput").ap()
    pos_t = nc.dram_tensor("pos", [PP, 4 * POSG], F32, kind="ExternalInput").ap()
    out_t = nc.dram_tensor("out", [1, 16], F32, kind="ExternalOutput").ap()

    with tile_mod.TileContext(nc) as tc:
        _body(nc, tc, big_t, pos_t, out_t)
    _split_sync_waits(nc)
    return nc


def _body(nc, tc, big_t, pos_t, out_t):
    from contextlib import ExitStack

    ctx = ExitStack()
    with ctx:
        const = ctx.enter_context(tc.tile_pool(name="const", bufs=1))
        work = ctx.enter_context(tc.tile_pool(name="work", bufs=1))
        psum = ctx.enter_context(tc.tile_pool(name="psum", bufs=1, space="PSUM"))

        # ---------- DMAs: everything per-partition-contiguous in ONE fat
        # tensor (DMA cost is ~30ns/packet + bytes/BW; one 7KB row per
        # partition per ring-half minimizes packets), halves split over the
        # two HWDGE rings; pos rides SWDGE ----------
        posf = const.tile([PP, 4, POSG], F32)
        nc.sync.dma_start(out=posf[:].rearrange("p a g -> p (a g)"), in_=pos_t)
        pos = posf[:]
        big = const.tile([P, 9 * PFD + 2 * T + 2 * CFD], F16)
        nc.sync.dma_start(out=big[:HP], in_=big_t[:HP])
        nc.scalar.dma_start(out=big[HP:], in_=big_t[HP:])
        pk = big[:, 0:9 * PFD].rearrange("p (c f) -> p c f", c=9)

        xo = big[:, 9 * PFD:9 * PFD + T]
        xoc = big[:, 9 * PFD + 2 * T:9 * PFD + 2 * T + CFD]
        npc = big[:, 9 * PFD + 2 * T + CFD:9 * PFD + 2 * T + 2 * CFD]

        stats = const.tile([P, 16], F32)
        nc.vector.memset(stats[:], 0.0)
        # cols: 0 giou, 1 spc, 2 pts, 3 sc1, 4 sc2, 5 spoA, 6 corrS, 7 corrN

        # ---------- ACT stream: pos-block transcendentals first (pos lands
        # ~3us before the big tensor), then the pair/plane exps, then ONE
        # table-set switch for the Ln ops ----------
        # s64 writes tanh(ptxy/2) into pos cols 6:8 (host left them zero), so
        # cxy_pt below reads one contiguous block.
        nc.scalar.activation(pos[:, :, 6:8], pos[:, :, 0:2], AF.Tanh,
                             scale=0.5)
        e_pt = work.tile([PP, 4, 4], F32)
        nc.scalar.activation(e_pt[:], pos[:, :, 2:6], AF.Exp)
        ec = work.tile([PP, 4, 80], F32)
        nc.scalar.activation(ec[:], pos[:, :, 98:178], AF.Exp)
        ep1 = work.tile([PP, 4, 1], F32)
        nc.scalar.activation(ep1[:], pos[:, :, 14:15], AF.Exp)
        thxy = work.tile([P, 2, PFD], F16)
        i_thxy = nc.scalar.activation(thxy[:], pk[:, 0:2, :], AF.Tanh,
                                      scale=0.5)
        ewh2 = work.tile([P, 2, PFD], F16)
        nc.scalar.activation(ewh2[:], pk[:, 2:4, :], AF.Exp)
        eo = work.tile([P, T], F32)
        nc.scalar.activation(eo[:], xo, AF.Exp)
        eoc = work.tile([P, CFD], F32)
        nc.scalar.activation(eoc[:], xoc, AF.Exp)
        # (natural_log_exp set loads here; the Ln ops are pinned after the
        # last Tanh so the scheduler cannot thrash the two table sets)
        spoc = work.tile([P, CFD], F16)
        i_spoc = nc.scalar.activation(spoc[:], eoc[:], AF.Ln, bias=1.0)
        add_dep_helper(i_spoc.ins, i_thxy.ins, False, "one table-set switch")
        spo = work.tile([P, T], F32)
        i_spo = nc.scalar.activation(spo[:], eo[:], AF.Ln, bias=1.0,
                                     accum_out=stats[:, 5:6])
        add_dep_helper(i_spo.ins, i_thxy.ins, False, "one table-set switch")
        sp1 = work.tile([PP, 4, 1], F32)
        i_sp1 = nc.scalar.activation(sp1[:], ep1[:], AF.Ln, bias=1.0,
                                     accum_out=stats[0:PP, 3:4])
        add_dep_helper(i_sp1.ins, i_thxy.ins, False, "one table-set switch")
        spc = work.tile([PP, 4, 80], F32)
        i_spc = nc.scalar.activation(spc[:], ec[:], AF.Ln, bias=1.0,
                                     accum_out=stats[0:PP, 1:2])
        add_dep_helper(i_spc.ins, i_thxy.ins, False, "one table-set switch")

        # ---------- pair chain (DVE, fp16, x/y fused at FD=2*PFD) ----------
        # Units: inter and areas are tracked /16 (rh carries a 1/16, nh3 a
        # -1/12 = -(4/3)/16) so every intermediate fits fp16 range; the host
        # pre-divides the CK threshold by 16 to match.
        s2 = work.tile([P, 2, PFD], F16)
        nc.vector.tensor_add(s2[:], thxy[:], ewh2[:])
        d2 = work.tile([P, 2, PFD], F16)
        nc.vector.tensor_sub(d2[:], thxy[:], ewh2[:])
        r1 = work.tile([P, 2, PFD], F16)
        nc.vector.tensor_tensor(r1[:], s2[:], pk[:, 4:6, :], op=OP.min)
        r2 = work.tile([P, 2, PFD], F16)
        nc.vector.tensor_max(r2[:], d2[:], pk[:, 6:8, :])
        n2 = work.tile([P, 2, PFD], F16)
        nc.vector.tensor_sub(n2[:], r2[:], r1[:])
        rh = work.tile([P, PFD], F16)
        nc.vector.tensor_scalar(
            out=rh[:], in0=n2[:, 1, :], scalar1=-1.0 / 16, scalar2=0.0,
            op0=OP.mult, op1=OP.max)
        ni = work.tile([P, PFD], F16)
        nc.vector.scalar_tensor_tensor(
            out=ni[:], in0=n2[:, 0, :], scalar=0.0, in1=rh[:],
            op0=OP.min, op1=OP.mult)
        nh3 = work.tile([P, PFD], F16)
        nc.vector.scalar_tensor_tensor(
            out=nh3[:], in0=ewh2[:, 0, :], scalar=-1.0 / 12, in1=ewh2[:, 1, :],
            op0=OP.mult, op1=OP.mult)
        mth = work.tile([P, PFD], F16)
        nc.vector.tensor_sub(mth[:], nh3[:], pk[:, 8, :])
        ind = work.tile([P, CFD, R], F16)
        nc.vector.tensor_tensor(
            ind[:].rearrange("p c r -> p (c r)"), ni[:], mth[:], op=OP.is_lt)
        ign = work.tile([P, CFD], F16)
        nc.vector.tensor_reduce(
            out=ign[:], in_=ind[:], axis=mybir.AxisListType.X, op=OP.max)

        # ---------- candidate corrections ----------
        t1 = work.tile([P, CFD], F16)
        nc.vector.scalar_tensor_tensor(
            out=t1[:], in0=ign[:], scalar=1.0, in1=npc,
            op0=OP.mult, op1=OP.mult, accum_out=stats[:, 7:8])
        t2 = work.tile([P, CFD], F16)
        nc.vector.scalar_tensor_tensor(
            out=t2[:], in0=t1[:], scalar=1.0, in1=spoc[:],
            op0=OP.mult, op1=OP.mult, accum_out=stats[:, 6:7])

        # ---------- positive-cell obj sums straight from the pos rows ----
        xp = work.tile([PP, 4, 1], F32)
        nc.vector.tensor_scalar(
            out=xp[:], in0=pos[:, :, 14:15], scalar1=1.0, scalar2=0.0,
            op0=OP.mult, op1=OP.add, accum_out=stats[0:PP, 4:5])

        # ---------- positive-cell block on GpSimd (idle engine) ----------
        # Works in 160th-of-image units with host-folded anchors, so e_pt IS
        # the half-width vector [hwp, hhp, hwt, hht]; areas are tracked in
        # quarter-units (hw*hh = area/4) which cancels in GIoU once EPS is
        # scaled by 160^2/4 = 6400.  Pool has no scalar_tensor_tensor, so
        # everything is tensor_tensor / tensor_scalar.
        g = nc.gpsimd
        cxy = work.tile([PP, 4, 4], F32)
        g.tensor_add(cxy[:], pos[:, :, 6:10], pos[:, :, 10:14])
        x1 = work.tile([PP, 4, 4], F32)
        g.tensor_sub(x1[:], cxy[:], e_pt[:])
        x2 = work.tile([PP, 4, 4], F32)
        g.tensor_add(x2[:], cxy[:], e_pt[:])
        # (Pool TT has no max/min -- these four comparisons run on DVE)
        imax = work.tile([PP, 4, 2], F32)
        nc.vector.tensor_max(imax[:], x1[:, :, 0:2], x1[:, :, 2:4])
        imin = work.tile([PP, 4, 2], F32)
        nc.vector.tensor_tensor(imin[:], x2[:, :, 0:2], x2[:, :, 2:4],
                                op=OP.min)
        emin = work.tile([PP, 4, 2], F32)
        nc.vector.tensor_tensor(emin[:], x1[:, :, 0:2], x1[:, :, 2:4],
                                op=OP.min)
        emax = work.tile([PP, 4, 2], F32)
        nc.vector.tensor_max(emax[:], x2[:, :, 0:2], x2[:, :, 2:4])
        iwh = work.tile([PP, 4, 2], F32)
        g.tensor_sub(iwh[:], imin[:], imax[:])
        iwr = work.tile([PP, 4, 2], F32)
        g.tensor_scalar(out=iwr[:], in0=iwh[:], scalar1=0.5, scalar2=0.0,
                        op0=OP.mult, op1=OP.max)
        inter = work.tile([PP, 4, 1], F32)
        g.tensor_mul(inter[:], iwr[:, :, 0:1], iwr[:, :, 1:2])   # inter/4
        aprod = work.tile([PP, 4, 2], F32)
        g.tensor_mul(aprod[:], e_pt[:, :, 0:4:2], e_pt[:, :, 1:4:2])
        uae = work.tile([PP, 4, 2], F32)
        g.tensor_add(uae[:, :, 0:1], aprod[:, :, 0:1], aprod[:, :, 1:2])
        g.tensor_sub(uae[:, :, 0:1], uae[:, :, 0:1], inter[:])   # union/4
        ewh = work.tile([PP, 4, 2], F32)
        g.tensor_sub(ewh[:], emax[:], emin[:])
        ewh5 = work.tile([PP, 4, 2], F32)
        g.tensor_scalar_mul(ewh5[:], ewh[:], 0.5)
        g.tensor_mul(uae[:, :, 1:2], ewh5[:, :, 0:1], ewh5[:, :, 1:2])
        dcu = work.tile([PP, 4, 1], F32)
        g.tensor_sub(dcu[:], uae[:, :, 1:2], uae[:, :, 0:1])
        uaeE = work.tile([PP, 4, 2], F32)
        g.tensor_scalar_add(uaeE[:], uae[:], 6400.0 * EPS)

        rr = work.tile([PP, 4, 2], F32)
        nc.vector.reciprocal(rr[:], uaeE[:])
        # giou_sum = n_pos - sum(iou) + sum(qv); host folds the constant
        iou = work.tile([PP, 4, 1], F32)
        nc.vector.scalar_tensor_tensor(
            out=iou[:], in0=inter[:], scalar=1.0, in1=rr[:, :, 0:1],
            op0=OP.mult, op1=OP.mult, accum_out=stats[0:PP, 0:1])
        qv = work.tile([PP, 4, 1], F32)
        nc.vector.scalar_tensor_tensor(
            out=qv[:], in0=dcu[:], scalar=1.0, in1=rr[:, :, 1:2],
            op0=OP.mult, op1=OP.mult, accum_out=stats[0:PP, 8:9])
        ptS = work.tile([PP, 4, 80], F32)
        nc.vector.scalar_tensor_tensor(
            out=ptS[:], in0=pos[:, :, 98:178], scalar=1.0,
            in1=pos[:, :, 18:98],
            op0=OP.mult, op1=OP.mult, accum_out=stats[0:PP, 2:3])

        # ---------- final partition reduction + output ----------
        ones = const.tile([P, 1], F32)
        nc.vector.memset(ones[:], 1.0)
        pst = psum.tile([1, 16], F32)
        nc.tensor.matmul(pst[:], ones[:], stats[:], start=True, stop=True)
        res = const.tile([1, 16], F32)
        nc.scalar.copy(res[:], pst[:])
        nc.sync.dma_start(out=out_t, in_=res[:])


def _host_prep(preds, targets):
    """Build per-core input maps from the full inputs (indexing/layout only)."""
    preds = np.ascontiguousarray(preds, np.float32)
    targets = np.ascontiguousarray(targets, np.float32)
    assert preds.shape == (B, A, H, W, C), preds.shape

    j = np.arange(CELLS)
    a = j // (H * W)
    rem = j % (H * W)
    gy = (rem // W).astype(np.float32)
    gx = (rem % W).astype(np.float32)
    aw = ANCHORS[a, 0]
    ah = ANCHORS[a, 1]
    lnaw8 = np.log(aw / 8.0).astype(np.float32)
    lnah8 = np.log(ah / 8.0).astype(np.float32)

    pf = preds.reshape(B, CELLS, C)
    tf = targets.reshape(B, CELLS, C)
    HW2 = H * W

    def plane2(x0, x1):
        return np.concatenate([x0.reshape(HP, T), x1.reshape(HP, T)], 0)

    in_maps = []
    for c in range(NCORES):
        i0 = BPC * c
        NP = CAND_N * R
        pk = np.zeros((9, NP), np.float32)
        pk[8] = 30000.0  # padding threshold, far beyond any real |ni|/16
        ckv = np.zeros((2, CAND_N), np.float32)
        pos = np.zeros((2 * M, POSG), np.float32)

        ci_base = 0
        for ii in range(BPC):
            b = i0 + ii
            to = tf[b, :, 4]
            idx = np.nonzero(to > 0)[0]
            assert len(idx) == M, len(idx)
            tb = tf[b][idx]
            tb64 = tb.astype(np.float64)
            gxk = gx[idx].astype(np.float64)
            gyk = gy[idx].astype(np.float64)
            CX160 = 2.0 * (tb64[:, 0] + gxk)
            CY160 = 2.0 * (tb64[:, 1] + gyk)
            HW160 = aw[idx] * np.exp(tb64[:, 2]) / 8.0
            HH160 = ah[idx] * np.exp(tb64[:, 3]) / 8.0
            CK3 = (4.0 * HW160 * HH160 + 25600.0 * EPS) / 3.0

            # candidate rectangles (2D grid), multiplicity and slot layout
            CXn = CX160 / 160.0
            CYn = CY160 / 160.0
            HWn = HW160 / 160.0
            HHn = HH160 / 160.0
            x0r = np.maximum(0, np.floor(80 * (CXn - HWn)).astype(np.int64))
            x1r = np.minimum(W - 1, np.ceil(80 * (CXn + HWn)).astype(np.int64))
            y0r = np.maximum(0, np.floor(80 * (CYn - HHn)).astype(np.int64))
            y1r = np.minimum(H - 1, np.ceil(80 * (CYn + HHn)).astype(np.int64))
            mult = np.zeros(HW2, np.int64)
            rec_cell = []
            rec_k = []
            rec_slot = []
            for k in range(M):
                yy, xx = np.meshgrid(
                    np.arange(y0r[k], y1r[k] + 1),
                    np.arange(x0r[k], x1r[k] + 1), indexing="ij")
                cells2d = (yy * W + xx).ravel()
                rec_cell.append(cells2d)
                rec_k.append(np.full(len(cells2d), k, np.int64))
                rec_slot.append(mult[cells2d].copy())
                mult[cells2d] += 1
            rec_cell = np.concatenate(rec_cell)
            rec_k = np.concatenate(rec_k)
            rec_slot = np.concatenate(rec_slot)
            assert mult.max() <= R, mult.max()

            cand2d = np.nonzero(mult > 0)[0]
            ncand2d = len(cand2d)
            crank = np.full(HW2, -1, np.int64)
            crank[cand2d] = np.arange(ncand2d)
            assert ci_base + 3 * ncand2d <= CAND_N

            for aa in range(3):
                ci = ci_base + aa * ncand2d + np.arange(ncand2d)
                cells = aa * HW2 + cand2d
                ckv[0, ci] = pf[b, cells, 4]
                ckv[1, ci] = 1.0 - tf[b, cells, 4]

            for aa in range(3):
                ci = ci_base + aa * ncand2d + crank[rec_cell]
                pidx = ci * R + rec_slot
                cells = aa * HW2 + rec_cell
                g1x = 2.0 * gx[cells] + 1.0
                g1y = 2.0 * gy[cells] + 1.0
                pk[0, pidx] = pf[b, cells, 0]
                pk[1, pidx] = pf[b, cells, 1]
                pk[2, pidx] = pf[b, cells, 2] + lnaw8[cells]
                pk[3, pidx] = pf[b, cells, 3] + lnah8[cells]
                pk[4, pidx] = (CX160[rec_k] + HW160[rec_k]) - g1x
                pk[5, pidx] = (CY160[rec_k] + HH160[rec_k]) - g1y
                pk[6, pidx] = (CX160[rec_k] - HW160[rec_k]) - g1x
                pk[7, pidx] = (CY160[rec_k] - HH160[rec_k]) - g1y
                pk[8, pidx] = CK3[rec_k] / 16.0
            ci_base += 3 * ncand2d

            # positive rows (packed layout in 160-units, see _body)
            r = slice(M * ii, M * (ii + 1))
            pos[r, 0:2] = pf[b][idx][:, 0:2]              # pred tx, ty
            pos[r, 2] = pf[b][idx][:, 2] + lnaw8[idx]     # exp -> hwp160
            pos[r, 3] = pf[b][idx][:, 3] + lnah8[idx]
            pos[r, 4] = tb[:, 2] + lnaw8[idx]             # exp -> hwt160
            pos[r, 5] = tb[:, 3] + lnah8[idx]
            # cols 6:8 left zero (device writes tanh there)
            pos[r, 8:10] = 2.0 * tb[:, 0:2]               # 2*ttx, 2*tty
            pos[r, 10] = 2.0 * gx[idx] + 1.0
            pos[r, 11] = 2.0 * gy[idx] + 1.0
            pos[r, 12] = 2.0 * gx[idx]
            pos[r, 13] = 2.0 * gy[idx]
            pos[r, 14] = pf[b][idx][:, 4]                 # obj logit
            pos[r, 18:98] = tb[:, 5:85]                   # tgt cls
            pos[r, 98:178] = pf[b][idx][:, 5:85]          # pred cls logits

        plane = np.concatenate([
            plane2(pf[i0, :, 4], pf[i0 + 1, :, 4]),
            plane2(tf[i0, :, 4], tf[i0 + 1, :, 4]),
            ckv[0].reshape(P, CFD),
            ckv[1].reshape(P, CFD),
        ], axis=1).astype(np.float32)  # [128, 684]

        big = np.concatenate(
            [pk.reshape(9, P, PFD).transpose(1, 0, 2).reshape(P, 9 * PFD),
             plane], axis=1)
        in_maps.append({
            "big": np.ascontiguousarray(big.astype(np.float16)),
            "pos": np.ascontiguousarray(pos.reshape(PP, 4 * POSG)),
        })
    return in_maps


def _combine(outs):
    s = np.sum(np.stack([o["out"].ravel() for o in outs]), axis=0,
               dtype=np.float64)
    n_pos = float(B * M)
    giou_val = (n_pos - s[0] + s[8]) / (n_pos + EPS)
    cls_val = (s[1] - s[2]) / (n_pos + EPS)
    pos_obj = s[3] - s[4]
    neg_obj = (s[5] - s[3]) - s[6]
    n_neg = B * (CELLS - M) - s[7]
    obj_val = (5.0 * pos_obj + neg_obj) / (5.0 * n_pos + n_neg + EPS)
    total = giou_val + obj_val + cls_val
    return np.array([total, giou_val, obj_val, cls_val], np.float32)


def kernel(preds, targets):
    global LAST_EXEC_NS, LAST_RESULT, _NC_CACHE
    in_maps = _host_prep(preds, targets)
    if _NC_CACHE is None:
        _NC_CACHE = _build_nc()
    nc = _NC_CACHE
    trace = os.environ.get("CCK_TRACE") == "1"
    res = None
    if trace:
        try:
            res = bass_utils.run_bass_kernel_spmd(
                nc, in_maps, core_ids=list(range(NCORES)), trace=True)
            LAST_EXEC_NS = res.exec_time_ns
        except Exception as e:
            print(f"[kernel] traced run failed ({e!r}); retrying untraced",
                  file=sys.stderr)
            res = None
    if res is None:
        res = bass_utils.run_bass_kernel_spmd(
            nc, in_maps, core_ids=list(range(NCORES)), trace=False)
    LAST_RESULT = res
    return _combine(res.results)
